# revision 1
# baseline (speedup 1.0000x reference)
"""Trainium2 Bass kernel for GeneralizedRingAttractorNoGain.

Computation (per reference):
  r0 = fixed bump (angle=pi), Wd7[i,j] = cos(2pi(i-j)/N)
  scan over t: rec = J0*sum(r) + J1*(r@Wo) + einsum('bn,anm,ba->bm', r, Wa, a_t)
               r = (1-ALPHA)*r + ALPHA*relu(rec)
  bump = stacked r;  r_delta7 = bump @ Wd7;  r_history = r_delta7 / max(r_delta7, axis=2)

Strategy: data-parallel over batch (8 cores x 8 rows).  All 34 weight
blocks (32 Wa + J1*Wo + J0*ones) are concatenated into Wcat resident in
SBUF; each step runs one matmul chain rec = sT.T @ Wcat_flat where
sT[(blk,n),b] = acat[b,blk] * r[b,n] is built on the vector engine from
the transposed state rT and a per-step broadcast action tile.  State is
kept transposed (rT) via a PE transpose of rec each step.
"""

import numpy as np

import concourse.bass as bass
import concourse.mybir as mybir
from concourse.bass import AP
from concourse.bass_utils import run_bass_kernel_spmd

N = 256
A = 32
B = 64
T_FULL = 128
NC = 8          # cores
BL = B // NC    # local batch = 8
J0 = -0.1
J1 = 0.1
ALPHA = 0.15
NBLK = 34       # 32 Wa + Wo + ones
F32 = mybir.dt.float32

_NC_CACHE = {}


def build_nc(T):
    nc = bass.Bass("TRN2", target_bir_lowering=False, debug=False, num_devices=NC, detect_race_conditions=False)

    # ---------------- DRAM I/O ----------------
    # Wcat chunks laid out [2(half), NBLK, 128, 256]
    wcat_d = nc.dram_tensor("wcat", [2, NBLK, 128, N], F32, kind="ExternalInput")
    # action tile per step, compact: [T, NBLK*BL]  (blk-major, b minor)
    ac_d = nc.dram_tensor("ac", [T, NBLK * BL], F32, kind="ExternalInput")
    # initial transposed state [128, 2, BL]
    r0t_d = nc.dram_tensor("r0t", [128, 2, BL], F32, kind="ExternalInput")
    # Wd7 halves [2, 128, 256]
    wd7_d = nc.dram_tensor("wd7", [2, 128, N], F32, kind="ExternalInput")
    # identity [128, 128]
    id_d = nc.dram_tensor("ident", [128, 128], F32, kind="ExternalInput")
    # outputs
    bump_d = nc.dram_tensor("bump_out", [BL, T, N], F32, kind="ExternalOutput")
    hist_d = nc.dram_tensor("hist_out", [BL, T, N], F32, kind="ExternalOutput")

    # ---------------- SBUF ----------------
    wcat = nc.alloc_sbuf_tensor("wcat_sb", [128, 2, NBLK, N], F32)      # 68KB/part
    a_sb = nc.alloc_sbuf_tensor("a_sb", [128, 4, NBLK * BL], F32)       # 4 bufs
    st = nc.alloc_sbuf_tensor("st_sb", [128, 2, 2, NBLK, BL], F32)      # dbl buf
    rt = nc.alloc_sbuf_tensor("rt_sb", [128, 2, BL], F32)
    ht = nc.alloc_sbuf_tensor("ht_sb", [128, 2, BL], F32)
    bumpT = nc.alloc_sbuf_tensor("bumpT_sb", [128, 2, BL, T], F32)
    rec_row = nc.alloc_sbuf_tensor("rec_row", [BL, N], F32)
    ident = nc.alloc_sbuf_tensor("ident_sb", [128, 128], F32)
    wd7 = nc.alloc_sbuf_tensor("wd7_sb", [128, 2, N], F32)
    brow = nc.alloc_sbuf_tensor("brow_sb", [128, 2, N], F32)            # dbl buf bump rows
    hrow = nc.alloc_sbuf_tensor("hrow_sb", [128, 2, N], F32)            # dbl buf hist rows
    mx = nc.alloc_sbuf_tensor("mx_sb", [128, 2], F32)
    rmx = nc.alloc_sbuf_tensor("rmx_sb", [128, 2], F32)

    # pitches (elements per partition)
    P_WCAT = 2 * NBLK * N
    P_A = 4 * NBLK * BL
    P_ST = 2 * 2 * NBLK * BL
    P_RT = 2 * BL
    P_BT = 2 * BL * T

    KCH = 2 * NBLK  # 68 matmul chunks per step

    import contextlib
    ctx = contextlib.ExitStack()
    psum_rec = ctx.enter_context(nc.psum_tensor("ps_rec", [BL, N], F32))
    psum_rt = ctx.enter_context(nc.psum_tensor("ps_rt", [128, 2 * BL], F32))
    psum_tb = ctx.enter_context(nc.psum_tensor("ps_tb", [128, 2, 128], F32))
    psum_d7a = ctx.enter_context(nc.psum_tensor("ps_d7a", [128, N], F32))
    psum_d7b = ctx.enter_context(nc.psum_tensor("ps_d7b", [128, N], F32))
    psum_d7s = [psum_d7a, psum_d7b]

    with (
        ctx,
        nc.Block() as block,
        nc.semaphore("s_boot") as s_boot,
        nc.semaphore("s_a") as s_a,
        nc.semaphore("s_st") as s_st,
        nc.semaphore("s_rec") as s_rec,
        nc.semaphore("s_row") as s_row,
        nc.semaphore("s_rt") as s_rt,
        nc.semaphore("s_h") as s_h,
        nc.semaphore("s_up") as s_up,
        nc.semaphore("s_tb") as s_tb,
        nc.semaphore("s_br") as s_br,
        nc.semaphore("s_d7") as s_d7,
        nc.semaphore("s_hr") as s_hr,
        nc.semaphore("s_odma") as s_odma,
        nc.semaphore("s_dve") as s_dve,
    ):
        # ================= SYNC: boot DMAs + action prefetch =================
        @block.sync
        def _(sync):
            # wcat: dram [2, NBLK, 128, 256] -> sbuf [128][2, NBLK, 256]
            sync.dma_start(
                out=wcat.ap(),
                in_=AP(wcat_d, 0, [[N, 128], [NBLK * 128 * N, 2], [128 * N, NBLK], [1, N]]),
            ).then_inc(s_boot, 16)
            # wd7: dram [2, 128, 256] -> sbuf [128][2, 256]
            sync.dma_start(
                out=wd7.ap(),
                in_=AP(wd7_d, 0, [[N, 128], [128 * N, 2], [1, N]]),
            ).then_inc(s_boot, 16)
            sync.dma_start(out=rt.ap(), in_=r0t_d.ap()).then_inc(s_boot, 16)
            sync.dma_start(out=ident.ap(), in_=id_d.ap()).then_inc(s_boot, 16)
            # action tiles: [1, 272] replicated to [128, 272]
            for t in range(T):
                if t >= 4:
                    sync.wait_ge(s_st, 2 * (t - 3))
                if t >= 1:
                    sync.wait_ge(s_a, 16 * t)
                sync.dma_start(
                    out=AP(a_sb, (t % 4) * NBLK * BL, [[P_A, 128], [1, NBLK * BL]]),
                    in_=AP(ac_d, t * NBLK * BL, [[0, 128], [1, NBLK * BL]]),
                ).then_inc(s_a, 16)
            # ---- endgame DMAs ----
            for b in range(BL):
                sync.wait_ge(s_br, b + 1)
                if b >= 1:
                    sync.wait_ge(s_odma, 16 * (2 * b - 1))
                sync.dma_start(
                    out=AP(bump_d, b * T * N, [[N, T], [1, N]]),
                    in_=AP(brow, (b % 2) * N, [[2 * N, T], [1, N]]),
                ).then_inc(s_odma, 16)
                sync.wait_ge(s_hr, b + 1)
                sync.wait_ge(s_odma, 16 * (2 * b + 1))
                sync.dma_start(
                    out=AP(hist_d, b * T * N, [[N, T], [1, N]]),
                    in_=AP(hrow, (b % 2) * N, [[2 * N, T], [1, N]]),
                ).then_inc(s_odma, 16)

        # ================= DVE: sT build, state update =================
        @block.vector
        def _(vector):
            vector.wait_ge(s_boot, 64)
            for t in range(T):
                vector.wait_ge(s_a, 16 * (t + 1))
                if t >= 2:
                    vector.wait_ge(s_rec, t - 1)  # st buf reuse
                buf = t % 2
                for h in range(2):
                    vector.tensor_mul(
                        AP(st, buf * 2 * NBLK * BL + h * NBLK * BL,
                           [[P_ST, 128], [BL, NBLK], [1, BL]]),
                        AP(rt, h * BL, [[P_RT, 128], [0, NBLK], [1, BL]]),
                        AP(a_sb, (t % 4) * NBLK * BL, [[P_A, 128], [BL, NBLK], [1, BL]]),
                    ).then_inc(s_st, 1)
                # state update: rt = 0.85*rt + ht
                vector.wait_ge(s_h, t + 1)
                vector.scalar_tensor_tensor(
                    AP(rt, 0, [[P_RT, 128], [1, 2 * BL]]),
                    AP(rt, 0, [[P_RT, 128], [1, 2 * BL]]),
                    1.0 - ALPHA,
                    AP(ht, 0, [[P_RT, 128], [1, 2 * BL]]),
                    op0=mybir.AluOpType.mult,
                    op1=mybir.AluOpType.add,
                ).then_inc(s_dve, 1)
                vector.wait_ge(s_dve, t + 1)
                # bumpT[:, h, b, t] = rt
                vector.tensor_copy(
                    AP(bumpT, t, [[P_BT, 128], [BL * T, 2], [T, BL]]),
                    AP(rt, 0, [[P_RT, 128], [BL, 2], [1, BL]]),
                ).then_inc(s_up, 1)
            # ---- endgame: normalize hist rows ----
            for b in range(BL):
                vector.wait_ge(s_d7, b + 1)
                pb = b % 2
                vector.tensor_reduce(
                    AP(mx, pb, [[2, T], [1, 1]]),
                    AP(psum_d7s[pb], 0, [[N, T], [1, N]]),
                    axis=mybir.AxisListType.X,
                    op=mybir.AluOpType.max,
                ).then_inc(s_dve, 1)
                vector.wait_ge(s_dve, T + 2 * b + 1)
                vector.reciprocal(
                    AP(rmx, pb, [[2, T], [1, 1]]),
                    AP(mx, pb, [[2, T], [1, 1]]),
                ).then_inc(s_dve, 1)
                vector.wait_ge(s_dve, T + 2 * b + 2)
                if b >= 2:
                    vector.wait_ge(s_odma, 16 * (2 * (b - 2) + 2))
                vector.tensor_scalar_mul(
                    AP(hrow, pb * N, [[2 * N, T], [1, N]]),
                    AP(psum_d7s[pb], 0, [[N, T], [1, N]]),
                    AP(rmx, pb, [[2, T], [1, 1]]),
                ).then_inc(s_hr, 1)

        # ================= PE: matmuls + transposes =================
        @block.tensor
        def _(tensor):
            tensor.wait_ge(s_boot, 64)
            for t in range(T):
                buf = t % 2
                tensor.wait_ge(s_st, 2 * t + 2)
                if t >= 1:
                    tensor.wait_ge(s_row, t)  # psum_rec consumed
                for k in range(KCH):
                    h, blk = k // NBLK, k % NBLK
                    inst = tensor.matmul(
                        psum_rec.ap(),
                        AP(st, buf * 2 * NBLK * BL + h * NBLK * BL + blk * BL,
                           [[P_ST, 128], [1, BL]]),
                        AP(wcat, h * NBLK * N + blk * N, [[P_WCAT, 128], [1, N]]),
                        start=(k == 0),
                        stop=(k == KCH - 1),
                    )
                    if k == KCH - 1:
                        inst.then_inc(s_rec, 1)
                # transpose rec_row halves -> psum_rt
                if t >= 1:
                    tensor.wait_ge(s_h, t)  # psum_rt consumed by ACT
                tensor.wait_ge(s_row, t + 1)
                tensor.transpose(
                    AP(psum_rt, 0, [[2 * BL, 128], [1, BL]]),
                    AP(rec_row, 0, [[N, BL], [1, 128]]),
                    AP(ident, 0, [[128, BL], [1, BL]]),
                )
                tensor.transpose(
                    AP(psum_rt, BL, [[2 * BL, 128], [1, BL]]),
                    AP(rec_row, 128, [[N, BL], [1, 128]]),
                    AP(ident, 0, [[128, BL], [1, BL]]),
                ).then_inc(s_rt, 1)
            # ---- endgame ----
            tensor.wait_ge(s_up, T)
            for b in range(BL):
                # bump row transposes
                if b >= 1:
                    tensor.wait_ge(s_br, b)  # psum_tb consumed
                for h in range(2):
                    inst = tensor.transpose(
                        AP(psum_tb, h * 128, [[2 * 128, T], [1, 128]]),
                        AP(bumpT, h * BL * T + b * T, [[P_BT, 128], [1, T]]),
                        ident.ap(),
                    )
                    if h == 1:
                        inst.then_inc(s_tb, 1)
                # d7 matmuls
                if b >= 2:
                    tensor.wait_ge(s_hr, b - 1)  # psum_d7 buf consumed
                pb = b % 2
                tensor.matmul(
                    AP(psum_d7s[pb], 0, [[N, T], [1, N]]),
                    AP(bumpT, 0 * BL * T + b * T, [[P_BT, 128], [1, T]]),
                    AP(wd7, 0 * N, [[2 * N, 128], [1, N]]),
                    start=True, stop=False,
                )
                tensor.matmul(
                    AP(psum_d7s[pb], 0, [[N, T], [1, N]]),
                    AP(bumpT, 1 * BL * T + b * T, [[P_BT, 128], [1, T]]),
                    AP(wd7, 1 * N, [[2 * N, 128], [1, N]]),
                    start=False, stop=True,
                ).then_inc(s_d7, 1)

        # ================= ACT: psum copies + relu =================
        @block.scalar
        def _(scalar):
            scalar.wait_ge(s_boot, 64)
            for t in range(T):
                scalar.wait_ge(s_rec, t + 1)
                if t >= 1:
                    scalar.wait_ge(s_rt, t)  # rec_row consumed by PE transposes
                scalar.copy(
                    AP(rec_row, 0, [[N, BL], [1, N]]),
                    psum_rec.ap(),
                ).then_inc(s_row, 1)
                # relu(0.15 * recT) from psum_rt
                scalar.wait_ge(s_rt, t + 1)
                if t >= 1:
                    scalar.wait_ge(s_up, t)  # ht consumed by DVE
                scalar.activation(
                    AP(ht, 0, [[P_RT, 128], [1, 2 * BL]]),
                    AP(psum_rt, 0, [[2 * BL, 128], [1, 2 * BL]]),
                    mybir.ActivationFunctionType.Relu,
                    scale=float(ALPHA),
                ).then_inc(s_h, 1)
            # ---- endgame: psum_tb -> brow ----
            for b in range(BL):
                scalar.wait_ge(s_tb, b + 1)
                if b >= 2:
                    scalar.wait_ge(s_odma, 16 * (2 * (b - 2) + 1))
                scalar.copy(
                    AP(brow, (b % 2) * N, [[2 * N, T], [1, N]]),
                    AP(psum_tb, 0, [[2 * 128, T], [1, N]]),
                ).then_inc(s_br, 1)

    return nc


def _host_prep(action_signal, Wo, Wa, T):
    # Wcat [NBLK, N, N]
    wcat = np.empty((NBLK, N, N), dtype=np.float32)
    wcat[:A] = Wa
    wcat[A] = J1 * Wo
    wcat[A + 1] = J0 * np.ones((N, N), dtype=np.float32)
    # chunk layout [2, NBLK, 128, N]
    wcat_d = np.ascontiguousarray(
        wcat.reshape(NBLK, 2, 128, N).transpose(1, 0, 2, 3))

    # acat [B, T, NBLK]
    acat = np.concatenate(
        [action_signal[:, :T, :],
         np.ones((B, T, 2), dtype=np.float32)], axis=2)

    # r0 row
    idx = np.arange(N, dtype=np.float32)
    center = np.float32(np.pi) * N / (2.0 * np.float32(np.pi))
    d = np.abs(idx - center)
    dist = np.minimum(d, N - d)
    width = N / 10.0
    bump0 = np.exp(-(dist ** 2) / (2.0 * width ** 2)).astype(np.float32)
    bump0 = bump0 / np.float32(np.linalg.norm(bump0))
    r0t = np.ascontiguousarray(
        np.broadcast_to(bump0.reshape(2, 128).T[:, :, None], (128, 2, BL))
    ).astype(np.float32)

    # Wd7 halves
    ii = np.arange(N, dtype=np.float32)
    ang = 2.0 * np.pi * (ii[:, None] - ii[None, :]) / N
    wd7 = np.cos(ang).astype(np.float32)
    wd7_d = np.ascontiguousarray(wd7.reshape(2, 128, N))

    ident = np.eye(128, dtype=np.float32)

    in_maps = []
    for c in range(NC):
        ac_core = np.ascontiguousarray(
            acat[c * BL:(c + 1) * BL].transpose(1, 2, 0).reshape(T, NBLK * BL))
        in_maps.append({
            "wcat": wcat_d, "ac": ac_core, "r0t": r0t,
            "wd7": wd7_d, "ident": ident,
        })
    return in_maps


def run(action_signal, Wo, Wa, T=T_FULL, **run_kwargs):
    if T not in _NC_CACHE:
        _NC_CACHE[T] = build_nc(T)
    nc = _NC_CACHE[T]
    in_maps = _host_prep(np.asarray(action_signal, dtype=np.float32),
                         np.asarray(Wo, dtype=np.float32),
                         np.asarray(Wa, dtype=np.float32), T)
    res = run_bass_kernel_spmd(nc, in_maps, core_ids=list(range(NC)), **run_kwargs)
    hist = np.concatenate([r["hist_out"] for r in res.results], axis=0)
    bump = np.concatenate([r["bump_out"] for r in res.results], axis=0)
    return (hist, bump), res


def kernel(action_signal, Wo, Wa):
    (hist, bump), _ = run(action_signal, Wo, Wa, T=T_FULL)
    return hist, bump



# revision 9
# speedup vs baseline: 3903.5018x; 3903.5018x over previous
"""Trainium2 Bass kernel for GeneralizedRingAttractorNoGain.

Computation (per reference):
  r0 = fixed bump (angle=pi), Wd7[i,j] = cos(2pi(i-j)/N)
  scan over t: rec = J0*sum(r) + J1*(r@Wo) + einsum('bn,anm,ba->bm', r, Wa, a_t)
               r = (1-ALPHA)*r + ALPHA*relu(rec)
  bump = stacked r;  r_delta7 = bump @ Wd7;  r_history = r_delta7 / max(r_delta7, axis=2)

Strategy: data-parallel over batch (8 cores x 8 rows).  33 weight blocks
(32 Wa + [J1*Wo + J0*ones]) are concatenated into Wcat resident in SBUF;
each step runs one PSUM-accumulated matmul chain rec = sT.T @ Wcat where
sT[(blk,n),b] = acat[b,blk] * r[b,n] is built on the vector engine from
the transposed state rT.  Matmul operands are bitcast to float32r (TF32-
style PE mode, 1 cycle/row at free size 256 vs 4 for fp32).  Per-step
serial tail: ACT relu(ALPHA*rec)->SBUF row, PE transposes row into PSUM,
DVE state update + sT build.  Epilogue computes bump @ Wd7 and the
row-max normalization on-chip.
"""

import numpy as np

import concourse.bass as bass
import concourse.mybir as mybir
from concourse.bass import AP
from concourse.bass_utils import run_bass_kernel_spmd

N = 256
A = 32
B = 64
T_FULL = 128
NC = 8          # cores
BL = B // NC    # local batch = 8
J0 = -0.1
J1 = 0.1
ALPHA = 0.15
NBLK = 33       # 32 Wa + (J1*Wo + J0*ones)
F32 = mybir.dt.float32
F32R = mybir.dt.float32r

_NC_CACHE = {}


def build_nc(T):
    nc = bass.Bass("TRN2", target_bir_lowering=False, debug=False, num_devices=NC, detect_race_conditions=False)

    # ---------------- DRAM I/O ----------------
    # Wcat chunks laid out [2(half), NBLK, 128, 256]
    wcat_d = nc.dram_tensor("wcat", [2, NBLK, 128, N], F32, kind="ExternalInput")
    # action tile per step, compact: [T, NBLK*BL]  (blk-major, b minor)
    ac_d = nc.dram_tensor("ac", [T, NBLK * BL], F32, kind="ExternalInput")
    # initial transposed state [128, 2, BL]
    r0t_d = nc.dram_tensor("r0t", [128, 2, BL], F32, kind="ExternalInput")
    # Wd7 halves [2, 128, 256]
    wd7_d = nc.dram_tensor("wd7", [2, 128, N], F32, kind="ExternalInput")
    # identity [128, 128]
    id_d = nc.dram_tensor("ident", [128, 128], F32, kind="ExternalInput")
    # outputs
    bump_d = nc.dram_tensor("bump_out", [BL, T, N], F32, kind="ExternalOutput")
    hist_d = nc.dram_tensor("hist_out", [BL, T, N], F32, kind="ExternalOutput")

    # ---------------- SBUF ----------------
    wcat = nc.alloc_sbuf_tensor("wcat_sb", [128, 2, NBLK, N], F32)      # 66KB/part
    a_sb = nc.alloc_sbuf_tensor("a_sb", [128, 4, NBLK * BL], F32)       # 4 bufs
    st = nc.alloc_sbuf_tensor("st_sb", [128, 2, NBLK, BL], F32)
    rt = nc.alloc_sbuf_tensor("rt_sb", [128, 2, BL], F32)
    bumpT = nc.alloc_sbuf_tensor("bumpT_sb", [128, 2, BL, T], F32)
    relu_row = nc.alloc_sbuf_tensor("relu_row", [BL, N], F32)
    ident = nc.alloc_sbuf_tensor("ident_sb", [128, 128], F32)
    wd7 = nc.alloc_sbuf_tensor("wd7_sb", [128, 2, N], F32)
    brow = nc.alloc_sbuf_tensor("brow_sb", [128, 2, N], F32)            # dbl buf bump rows
    hrow = nc.alloc_sbuf_tensor("hrow_sb", [128, 2, N], F32)            # dbl buf hist rows
    mx = nc.alloc_sbuf_tensor("mx_sb", [128, 2], F32)
    rmx = nc.alloc_sbuf_tensor("rmx_sb", [128, 2], F32)

    # pitches (elements per partition)
    P_WCAT = 2 * NBLK * N
    P_A = 4 * NBLK * BL
    P_ST = 2 * NBLK * BL
    P_RT = 2 * BL
    P_BT = 2 * BL * T

    KCH = 2 * NBLK  # 66 matmul chunks per step

    import contextlib
    ctx = contextlib.ExitStack()
    psum_rec = ctx.enter_context(nc.psum_tensor("ps_rec", [BL, N], F32))
    psum_rt = ctx.enter_context(nc.psum_tensor("ps_rt", [128, 2 * BL], F32))
    psum_tb = ctx.enter_context(nc.psum_tensor("ps_tb", [128, 2, 128], F32))
    psum_d7a = ctx.enter_context(nc.psum_tensor("ps_d7a", [128, N], F32))
    psum_d7b = ctx.enter_context(nc.psum_tensor("ps_d7b", [128, N], F32))
    psum_d7s = [psum_d7a, psum_d7b]

    with (
        ctx,
        nc.Block() as block,
        nc.semaphore("s_boot") as s_boot,
        nc.semaphore("s_a") as s_a,
        nc.semaphore("s_st") as s_st,
        nc.semaphore("s_rec") as s_rec,
        nc.semaphore("s_hrow") as s_hrow,
        nc.semaphore("s_rt") as s_rt,
        nc.semaphore("s_up") as s_up,
        nc.semaphore("s_tb") as s_tb,
        nc.semaphore("s_br") as s_br,
        nc.semaphore("s_d7") as s_d7,
        nc.semaphore("s_hr") as s_hr,
        nc.semaphore("s_odma") as s_odma,
        nc.semaphore("s_v") as s_v,
    ):
        # ================= SYNC: boot DMAs + action prefetch =================
        @block.sync
        def _(sync):
            # wcat: dram [2, NBLK, 128, 256] -> sbuf [128][2, NBLK, 256]
            # (values pre-rounded to fp32r on host; bitcast marks the rounding)
            sync.dma_start(
                out=wcat.ap().bitcast(F32R),
                in_=AP(wcat_d, 0, [[N, 128], [NBLK * 128 * N, 2], [128 * N, NBLK], [1, N]]).bitcast(F32R),
            ).then_inc(s_boot, 16)
            # wd7: dram [2, 128, 256] -> sbuf [128][2, 256]
            sync.dma_start(
                out=wd7.ap(),
                in_=AP(wd7_d, 0, [[N, 128], [128 * N, 2], [1, N]]),
            ).then_inc(s_boot, 16)
            sync.dma_start(out=rt.ap(), in_=r0t_d.ap()).then_inc(s_boot, 16)
            sync.dma_start(out=ident.ap(), in_=id_d.ap()).then_inc(s_boot, 16)
            # action tiles: [1, 264] replicated to [128, 264]
            for t in range(T):
                if t >= 4:
                    sync.wait_ge(s_st, 2 * (t - 3))
                if t >= 1:
                    sync.wait_ge(s_a, 16 * t)
                sync.dma_start(
                    out=AP(a_sb, (t % 4) * NBLK * BL, [[P_A, 128], [1, NBLK * BL]]),
                    in_=AP(ac_d, t * NBLK * BL, [[0, 128], [1, NBLK * BL]]),
                ).then_inc(s_a, 16)
            # ---- endgame DMAs ----
            for b in range(BL):
                sync.wait_ge(s_br, b + 1)
                if b >= 1:
                    sync.wait_ge(s_odma, 16 * (2 * b - 1))
                sync.dma_start(
                    out=AP(bump_d, b * T * N, [[N, T], [1, N]]),
                    in_=AP(brow, (b % 2) * N, [[2 * N, T], [1, N]]),
                ).then_inc(s_odma, 16)
                sync.wait_ge(s_hr, b + 1)
                sync.wait_ge(s_odma, 16 * (2 * b + 1))
                sync.dma_start(
                    out=AP(hist_d, b * T * N, [[N, T], [1, N]]),
                    in_=AP(hrow, (b % 2) * N, [[2 * N, T], [1, N]]),
                ).then_inc(s_odma, 16)

        # ================= DVE: state update, sT build =================
        @block.vector
        def _(vector):
            nv = 0  # s_v self-sync counter (same-engine RAW drains)
            vector.wait_ge(s_boot, 64)
            for t in range(T):
                if t >= 1:
                    # rt = 0.85*rt + psum_rt (psum_rt = ALPHA*relu(rec_{t-1}), transposed)
                    vector.wait_ge(s_rt, t)
                    vector.scalar_tensor_tensor(
                        AP(rt, 0, [[P_RT, 128], [1, 2 * BL]]),
                        AP(rt, 0, [[P_RT, 128], [1, 2 * BL]]),
                        1.0 - ALPHA,
                        AP(psum_rt, 0, [[2 * BL, 128], [1, 2 * BL]]),
                        op0=mybir.AluOpType.mult,
                        op1=mybir.AluOpType.add,
                    ).then_inc(s_v, 1)
                    nv += 1
                    vector.wait_ge(s_v, nv)  # rt writes drained before builds read
                vector.wait_ge(s_a, 16 * (t + 1))
                for h in range(2):
                    vector.tensor_mul(
                        AP(st, h * NBLK * BL,
                           [[P_ST, 128], [BL, NBLK], [1, BL]]).bitcast(F32R),
                        AP(rt, h * BL, [[P_RT, 128], [0, NBLK], [1, BL]]),
                        AP(a_sb, (t % 4) * NBLK * BL, [[P_A, 128], [BL, NBLK], [1, BL]]),
                    ).then_inc(s_st, 1)
                if t >= 1:
                    # bumpT[:, h, b, t-1] = rt  (state after step t-1)
                    vector.tensor_copy(
                        AP(bumpT, t - 1, [[P_BT, 128], [BL * T, 2], [T, BL]]),
                        AP(rt, 0, [[P_RT, 128], [BL, 2], [1, BL]]),
                    ).then_inc(s_up, 1)
            # final update + last bump column
            vector.wait_ge(s_rt, T)
            vector.scalar_tensor_tensor(
                AP(rt, 0, [[P_RT, 128], [1, 2 * BL]]),
                AP(rt, 0, [[P_RT, 128], [1, 2 * BL]]),
                1.0 - ALPHA,
                AP(psum_rt, 0, [[2 * BL, 128], [1, 2 * BL]]),
                op0=mybir.AluOpType.mult,
                op1=mybir.AluOpType.add,
            ).then_inc(s_v, 1)
            nv += 1
            vector.wait_ge(s_v, nv)
            vector.tensor_copy(
                AP(bumpT, T - 1, [[P_BT, 128], [BL * T, 2], [T, BL]]),
                AP(rt, 0, [[P_RT, 128], [BL, 2], [1, BL]]),
            ).then_inc(s_up, 1)
            # ---- endgame: normalize hist rows ----
            for b in range(BL):
                vector.wait_ge(s_d7, b + 1)
                pb = b % 2
                vector.tensor_reduce(
                    AP(mx, pb, [[2, T], [1, 1]]),
                    AP(psum_d7s[pb], 0, [[N, T], [1, N]]),
                    axis=mybir.AxisListType.X,
                    op=mybir.AluOpType.max,
                ).then_inc(s_v, 1)
                nv += 1
                vector.wait_ge(s_v, nv)
                vector.reciprocal(
                    AP(rmx, pb, [[2, T], [1, 1]]),
                    AP(mx, pb, [[2, T], [1, 1]]),
                ).then_inc(s_v, 1)
                nv += 1
                vector.wait_ge(s_v, nv)
                if b >= 2:
                    vector.wait_ge(s_odma, 16 * (2 * (b - 2) + 2))
                vector.tensor_scalar_mul(
                    AP(hrow, pb * N, [[2 * N, T], [1, N]]),
                    AP(psum_d7s[pb], 0, [[N, T], [1, N]]),
                    AP(rmx, pb, [[2, T], [1, 1]]),
                ).then_inc(s_hr, 1)

        # ================= PE: matmuls + transposes =================
        @block.tensor
        def _(tensor):
            tensor.wait_ge(s_boot, 64)
            for t in range(T):
                for k in range(KCH):
                    h, blk = k // NBLK, k % NBLK
                    if k == 0:
                        tensor.wait_ge(s_st, 2 * t + 1)
                    elif k == NBLK:
                        tensor.wait_ge(s_st, 2 * t + 2)
                    inst = tensor.matmul(
                        psum_rec.ap(),
                        AP(st, h * NBLK * BL + blk * BL,
                           [[P_ST, 128], [1, BL]]).bitcast(F32R),
                        AP(wcat, h * NBLK * N + blk * N, [[P_WCAT, 128], [1, N]]).bitcast(F32R),
                        start=(k == 0),
                        stop=(k == KCH - 1),
                    )
                    if k == KCH - 1:
                        inst.then_inc(s_rec, 1)
                # transpose relu'd row halves -> psum_rt
                tensor.wait_ge(s_hrow, t + 1)
                tensor.transpose(
                    AP(psum_rt, 0, [[2 * BL, 128], [1, BL]]),
                    AP(relu_row, 0, [[N, BL], [1, 128]]),
                    AP(ident, 0, [[128, BL], [1, BL]]),
                )
                tensor.transpose(
                    AP(psum_rt, BL, [[2 * BL, 128], [1, BL]]),
                    AP(relu_row, 128, [[N, BL], [1, 128]]),
                    AP(ident, 0, [[128, BL], [1, BL]]),
                ).then_inc(s_rt, 1)
            # ---- endgame ----
            tensor.wait_ge(s_up, T)
            for b in range(BL):
                # bump row transposes
                if b >= 1:
                    tensor.wait_ge(s_br, b)  # psum_tb consumed
                for h in range(2):
                    inst = tensor.transpose(
                        AP(psum_tb, h * 128, [[2 * 128, T], [1, 128]]),
                        AP(bumpT, h * BL * T + b * T, [[P_BT, 128], [1, T]]),
                        ident.ap(),
                    )
                    if h == 1:
                        inst.then_inc(s_tb, 1)
                # d7 matmuls
                if b >= 2:
                    tensor.wait_ge(s_hr, b - 1)  # psum_d7 buf consumed
                pb = b % 2
                tensor.matmul(
                    AP(psum_d7s[pb], 0, [[N, T], [1, N]]),
                    AP(bumpT, 0 * BL * T + b * T, [[P_BT, 128], [1, T]]),
                    AP(wd7, 0 * N, [[2 * N, 128], [1, N]]),
                    start=True, stop=False,
                )
                tensor.matmul(
                    AP(psum_d7s[pb], 0, [[N, T], [1, N]]),
                    AP(bumpT, 1 * BL * T + b * T, [[P_BT, 128], [1, T]]),
                    AP(wd7, 1 * N, [[2 * N, 128], [1, N]]),
                    start=False, stop=True,
                ).then_inc(s_d7, 1)

        # ================= ACT: relu(ALPHA * rec) -> SBUF row =================
        @block.scalar
        def _(scalar):
            scalar.wait_ge(s_boot, 64)
            for t in range(T):
                scalar.wait_ge(s_rec, t + 1)
                scalar.activation(
                    AP(relu_row, 0, [[N, BL], [1, N]]),
                    psum_rec.ap(),
                    mybir.ActivationFunctionType.Relu,
                    scale=float(ALPHA),
                ).then_inc(s_hrow, 1)
            # ---- endgame: psum_tb -> brow ----
            for b in range(BL):
                scalar.wait_ge(s_tb, b + 1)
                if b >= 2:
                    scalar.wait_ge(s_odma, 16 * (2 * (b - 2) + 1))
                scalar.copy(
                    AP(brow, (b % 2) * N, [[2 * N, T], [1, N]]),
                    AP(psum_tb, 0, [[2 * 128, T], [1, N]]),
                ).then_inc(s_br, 1)

    return nc


def _round_fp32r(x):
    # round-to-nearest-even onto fp32r (1s + 8e + 11m, low 12 bits zero)
    u = x.view(np.uint32)
    u = (u + np.uint32(0x7FF) + ((u >> np.uint32(12)) & np.uint32(1))) & np.uint32(0xFFFFF000)
    return u.view(np.float32)


def _host_prep(action_signal, Wo, Wa, T):
    # Wcat [NBLK, N, N]
    wcat = np.empty((NBLK, N, N), dtype=np.float32)
    wcat[:A] = Wa
    wcat[A] = J1 * Wo + J0 * np.ones((N, N), dtype=np.float32)
    wcat = _round_fp32r(wcat)
    # chunk layout [2, NBLK, 128, N]
    wcat_d = np.ascontiguousarray(
        wcat.reshape(NBLK, 2, 128, N).transpose(1, 0, 2, 3))

    # acat [B, T, NBLK]
    acat = np.concatenate(
        [action_signal[:, :T, :],
         np.ones((B, T, 1), dtype=np.float32)], axis=2)

    # r0 row
    idx = np.arange(N, dtype=np.float32)
    center = np.float32(np.pi) * N / (2.0 * np.float32(np.pi))
    d = np.abs(idx - center)
    dist = np.minimum(d, N - d)
    width = N / 10.0
    bump0 = np.exp(-(dist ** 2) / (2.0 * width ** 2)).astype(np.float32)
    bump0 = bump0 / np.float32(np.linalg.norm(bump0))
    r0t = np.ascontiguousarray(
        np.broadcast_to(bump0.reshape(2, 128).T[:, :, None], (128, 2, BL))
    ).astype(np.float32)

    # Wd7 halves
    ii = np.arange(N, dtype=np.float32)
    ang = 2.0 * np.pi * (ii[:, None] - ii[None, :]) / N
    wd7 = np.cos(ang).astype(np.float32)
    wd7_d = np.ascontiguousarray(wd7.reshape(2, 128, N))

    ident = np.eye(128, dtype=np.float32)

    in_maps = []
    for c in range(NC):
        ac_core = np.ascontiguousarray(
            acat[c * BL:(c + 1) * BL].transpose(1, 2, 0).reshape(T, NBLK * BL))
        in_maps.append({
            "wcat": wcat_d, "ac": ac_core, "r0t": r0t,
            "wd7": wd7_d, "ident": ident,
        })
    return in_maps


def run(action_signal, Wo, Wa, T=T_FULL, **run_kwargs):
    if T not in _NC_CACHE:
        _NC_CACHE[T] = build_nc(T)
    nc = _NC_CACHE[T]
    in_maps = _host_prep(np.asarray(action_signal, dtype=np.float32),
                         np.asarray(Wo, dtype=np.float32),
                         np.asarray(Wa, dtype=np.float32), T)
    res = run_bass_kernel_spmd(nc, in_maps, core_ids=list(range(NC)), **run_kwargs)
    hist = np.concatenate([r["hist_out"] for r in res.results], axis=0)
    bump = np.concatenate([r["bump_out"] for r in res.results], axis=0)
    return (hist, bump), res


def kernel(action_signal, Wo, Wa):
    (hist, bump), _ = run(action_signal, Wo, Wa, T=T_FULL)
    return hist, bump


# revision 13
# speedup vs baseline: 4993.7258x; 1.2793x over previous
"""Trainium2 Bass kernel for GeneralizedRingAttractorNoGain.

Computation (per reference):
  r0 = fixed bump (angle=pi), Wd7[i,j] = cos(2pi(i-j)/N)
  scan over t: rec = J0*sum(r) + J1*(r@Wo) + einsum('bn,anm,ba->bm', r, Wa, a_t)
               r = (1-ALPHA)*r + ALPHA*relu(rec)
  bump = stacked r;  r_delta7 = bump @ Wd7;  r_history = r_delta7 / max(r_delta7, axis=2)

Strategy: data-parallel over batch (8 cores x 8 rows).  33 weight blocks
(32 Wa + [J1*Wo + J0*ones]) are concatenated into Wcat (bf16) resident in
SBUF; each step streams Wcat through the PE as the moving operand of a
PSUM-accumulated chain rec = sT.T @ Wcat with stationary sT[(blk,n),b] =
acat[b,blk] * r[b,n] built on the vector engine.

The chain is split into two m-halves (bf16 runs 1 cycle/row at any free
size) so the serial tail of step t hides under PE work:
  chain_m0(t) -> ACT relu_h0 -> [T_h0 interleaved in chain_m1(t)]
  chain_m1(t) -> ACT relu_h1 -> [T_h1 interleaved in chain_m0(t+1)]
The state update r_{t+1} = 0.85*r_t + ALPHA*relu(rec_t)^T is formed
directly in PSUM: an identity matmul re-injects 0.85*r_t (f32r), then the
relu'd half transposes accumulate on top.  DVE only builds sT (bf16) and
copies state out; nothing else sits on the critical path.
"""

import numpy as np
import ml_dtypes

import concourse.bass as bass
import concourse.mybir as mybir
from concourse.bass import AP
from concourse.bass_utils import run_bass_kernel_spmd

N = 256
A = 32
B = 64
T_FULL = 128
NC = 8          # cores
BL = B // NC    # local batch = 8
J0 = -0.1
J1 = 0.1
ALPHA = 0.15
NBLK = 33       # 32 Wa + (J1*Wo + J0*ones)
F32 = mybir.dt.float32
F32R = mybir.dt.float32r
BF16 = mybir.dt.bfloat16

INS = 20        # chunk index where the transpose pair interleaves

_NC_CACHE = {}


def build_nc(T):
    nc = bass.Bass("TRN2", target_bir_lowering=False, debug=False, num_devices=NC, detect_race_conditions=False)

    # ---------------- DRAM I/O ----------------
    # Wcat chunks laid out [2(half), NBLK, 128, 256] in bf16
    wcat_d = nc.dram_tensor("wcat", [2, NBLK, 128, N], BF16, kind="ExternalInput")
    # action tile per step, compact: [T, NBLK*BL]  (blk-major, b minor)
    ac_d = nc.dram_tensor("ac", [T, NBLK * BL], F32, kind="ExternalInput")
    # initial transposed state [128, 2, BL] (fp32r-rounded on host)
    r0t_d = nc.dram_tensor("r0t", [128, 2, BL], F32, kind="ExternalInput")
    # Wd7 halves [2, 128, 256]
    wd7_d = nc.dram_tensor("wd7", [2, 128, N], F32, kind="ExternalInput")
    # identities: fp32 (endgame transposes), f32r copy (state re-injection),
    # bf16 8x8 (relu transposes)
    id_d = nc.dram_tensor("ident", [128, 128], F32, kind="ExternalInput")
    idr_d = nc.dram_tensor("identr", [128, 128], F32, kind="ExternalInput")
    idb_d = nc.dram_tensor("identb", [8, 8], BF16, kind="ExternalInput")
    # outputs
    bump_d = nc.dram_tensor("bump_out", [BL, T, N], F32, kind="ExternalOutput")
    hist_d = nc.dram_tensor("hist_out", [BL, T, N], F32, kind="ExternalOutput")

    # ---------------- SBUF ----------------
    wcat = nc.alloc_sbuf_tensor("wcat_sb", [128, 2, NBLK, N], BF16)     # 33.8KB/part
    a_sb = nc.alloc_sbuf_tensor("a_sb", [128, 4, NBLK * BL], F32)       # 4 bufs
    st = nc.alloc_sbuf_tensor("st_sb", [128, 2, 2, NBLK, BL], BF16)     # dbl buf
    rt = nc.alloc_sbuf_tensor("rt_sb", [128, 2, BL], F32)
    bumpT = nc.alloc_sbuf_tensor("bumpT_sb", [128, 2, BL, T], F32)
    relu_row = nc.alloc_sbuf_tensor("relu_row", [BL, N], BF16)
    ident = nc.alloc_sbuf_tensor("ident_sb", [128, 128], F32)
    identr = nc.alloc_sbuf_tensor("identr_sb", [128, 128], F32)
    identb = nc.alloc_sbuf_tensor("identb_sb", [8, 8], BF16)
    wd7 = nc.alloc_sbuf_tensor("wd7_sb", [128, 2, N], F32)
    brow = nc.alloc_sbuf_tensor("brow_sb", [128, 2, N], F32)            # dbl buf bump rows
    hrow = nc.alloc_sbuf_tensor("hrow_sb", [128, 2, N], F32)            # dbl buf hist rows
    mx = nc.alloc_sbuf_tensor("mx_sb", [128, 2], F32)
    rmx = nc.alloc_sbuf_tensor("rmx_sb", [128, 2], F32)

    # pitches (elements per partition)
    P_WCAT = 2 * NBLK * N
    P_A = 4 * NBLK * BL
    P_ST = 2 * 2 * NBLK * BL
    P_RT = 2 * BL
    P_BT = 2 * BL * T

    KCH = 2 * NBLK  # 66 K-chunks per half-chain

    import contextlib
    ctx = contextlib.ExitStack()
    ps_rec = [ctx.enter_context(nc.psum_tensor(f"ps_rec{m}", [BL, 128], F32))
              for m in range(2)]
    ps_rt = [ctx.enter_context(nc.psum_tensor(f"ps_rt{h}", [128, BL], F32))
             for h in range(2)]
    psum_tb = ctx.enter_context(nc.psum_tensor("ps_tb", [128, 2, 128], F32))
    psum_d7a = ctx.enter_context(nc.psum_tensor("ps_d7a", [128, N], F32))
    psum_d7b = ctx.enter_context(nc.psum_tensor("ps_d7b", [128, N], F32))
    psum_d7s = [psum_d7a, psum_d7b]

    with (
        ctx,
        nc.Block() as block,
        nc.semaphore("s_boot") as s_boot,
        nc.semaphore("s_a") as s_a,
        nc.semaphore("s_st") as s_st,
        nc.semaphore("s_rec") as s_rec,
        nc.semaphore("s_hrow0") as s_hrow0,
        nc.semaphore("s_hrow1") as s_hrow1,
        nc.semaphore("s_rt0") as s_rt0,
        nc.semaphore("s_rt1") as s_rt1,
        nc.semaphore("s_sm") as s_sm,
        nc.semaphore("s_up") as s_up,
        nc.semaphore("s_tb") as s_tb,
        nc.semaphore("s_br") as s_br,
        nc.semaphore("s_d7") as s_d7,
        nc.semaphore("s_hr") as s_hr,
        nc.semaphore("s_odma") as s_odma,
        nc.semaphore("s_v") as s_v,
    ):
        # ================= SYNC: boot DMAs + action prefetch =================
        @block.sync
        def _(sync):
            # wcat: dram [2, NBLK, 128, 256] -> sbuf [128][2, NBLK, 256]
            sync.dma_start(
                out=wcat.ap(),
                in_=AP(wcat_d, 0, [[N, 128], [NBLK * 128 * N, 2], [128 * N, NBLK], [1, N]]),
            ).then_inc(s_boot, 16)
            # wd7: dram [2, 128, 256] -> sbuf [128][2, 256]
            sync.dma_start(
                out=wd7.ap(),
                in_=AP(wd7_d, 0, [[N, 128], [128 * N, 2], [1, N]]),
            ).then_inc(s_boot, 16)
            sync.dma_start(out=rt.ap().bitcast(F32R),
                           in_=r0t_d.ap().bitcast(F32R)).then_inc(s_boot, 16)
            sync.dma_start(out=ident.ap(), in_=id_d.ap()).then_inc(s_boot, 16)
            sync.dma_start(out=identr.ap().bitcast(F32R),
                           in_=idr_d.ap().bitcast(F32R)).then_inc(s_boot, 16)
            sync.dma_start(out=identb.ap(), in_=idb_d.ap()).then_inc(s_boot, 16)
            # action tiles: [1, 264] replicated to [128, 264]
            for t in range(T):
                if t >= 4:
                    sync.wait_ge(s_st, 2 * (t - 3))
                if t >= 1:
                    sync.wait_ge(s_a, 16 * t)
                sync.dma_start(
                    out=AP(a_sb, (t % 4) * NBLK * BL, [[P_A, 128], [1, NBLK * BL]]),
                    in_=AP(ac_d, t * NBLK * BL, [[0, 128], [1, NBLK * BL]]),
                ).then_inc(s_a, 16)
            # ---- endgame DMAs ----
            for b in range(BL):
                sync.wait_ge(s_br, b + 1)
                if b >= 1:
                    sync.wait_ge(s_odma, 16 * (2 * b - 1))
                sync.dma_start(
                    out=AP(bump_d, b * T * N, [[N, T], [1, N]]),
                    in_=AP(brow, (b % 2) * N, [[2 * N, T], [1, N]]),
                ).then_inc(s_odma, 16)
                sync.wait_ge(s_hr, b + 1)
                sync.wait_ge(s_odma, 16 * (2 * b + 1))
                sync.dma_start(
                    out=AP(hist_d, b * T * N, [[N, T], [1, N]]),
                    in_=AP(hrow, (b % 2) * N, [[2 * N, T], [1, N]]),
                ).then_inc(s_odma, 16)

        # ================= DVE: sT builds + state copies =================
        @block.vector
        def _(vector):
            vector.wait_ge(s_boot, 96)

            def build(t, h, src_psum):
                src = (AP(ps_rt[h], 0, [[BL, 128], [0, NBLK], [1, BL]]) if src_psum
                       else AP(rt, h * BL, [[P_RT, 128], [0, NBLK], [1, BL]]))
                return vector.tensor_mul(
                    AP(st, (t % 2) * 2 * NBLK * BL + h * NBLK * BL,
                       [[P_ST, 128], [BL, NBLK], [1, BL]]),
                    src,
                    AP(a_sb, (t % 4) * NBLK * BL, [[P_A, 128], [BL, NBLK], [1, BL]]),
                )

            # t=0: builds from r0 (SBUF), then pre-scale rt by 0.85 in place
            vector.wait_ge(s_a, 16)
            build(0, 0, False).then_inc(s_st, 1)
            build(0, 1, False).then_inc(s_st, 1)
            vector.tensor_scalar_mul(
                AP(rt, 0, [[P_RT, 128], [1, BL]]).bitcast(F32R),
                AP(rt, 0, [[P_RT, 128], [1, BL]]),
                1.0 - ALPHA,
            )
            vector.tensor_scalar_mul(
                AP(rt, BL, [[P_RT, 128], [1, BL]]).bitcast(F32R),
                AP(rt, BL, [[P_RT, 128], [1, BL]]),
                1.0 - ALPHA,
            ).then_inc(s_sm, 1)
            for t in range(1, T):
                vector.wait_ge(s_rt0, t)       # ps_rt0 = r_t[h0]
                vector.wait_ge(s_a, 16 * (t + 1))
                build(t, 0, True).then_inc(s_st, 1)
                vector.wait_ge(s_rt1, t)       # ps_rt1 = r_t[h1]
                build(t, 1, True).then_inc(s_st, 1)
                # bumpT[:, h, b, t-1] = r_t
                for h in range(2):
                    inst = vector.tensor_copy(
                        AP(bumpT, h * BL * T + (t - 1), [[P_BT, 128], [T, BL]]),
                        AP(ps_rt[h], 0, [[BL, 128], [1, BL]]),
                    )
                inst.then_inc(s_up, 1)
                # rt = 0.85 * r_t (f32r) for the PE's identity re-injection
                vector.tensor_scalar_mul(
                    AP(rt, 0, [[P_RT, 128], [1, BL]]).bitcast(F32R),
                    AP(ps_rt[0], 0, [[BL, 128], [1, BL]]),
                    1.0 - ALPHA,
                )
                vector.tensor_scalar_mul(
                    AP(rt, BL, [[P_RT, 128], [1, BL]]).bitcast(F32R),
                    AP(ps_rt[1], 0, [[BL, 128], [1, BL]]),
                    1.0 - ALPHA,
                ).then_inc(s_sm, 1)
            # last bump column
            vector.wait_ge(s_rt0, T)
            vector.wait_ge(s_rt1, T)
            for h in range(2):
                inst = vector.tensor_copy(
                    AP(bumpT, h * BL * T + (T - 1), [[P_BT, 128], [T, BL]]),
                    AP(ps_rt[h], 0, [[BL, 128], [1, BL]]),
                )
            inst.then_inc(s_up, 1)
            # ---- endgame: normalize hist rows ----
            nv = 0  # s_v self-sync counter (same-engine RAW drains)
            for b in range(BL):
                vector.wait_ge(s_d7, b + 1)
                pb = b % 2
                vector.tensor_reduce(
                    AP(mx, pb, [[2, T], [1, 1]]),
                    AP(psum_d7s[pb], 0, [[N, T], [1, N]]),
                    axis=mybir.AxisListType.X,
                    op=mybir.AluOpType.max,
                ).then_inc(s_v, 1)
                nv += 1
                vector.wait_ge(s_v, nv)
                vector.reciprocal(
                    AP(rmx, pb, [[2, T], [1, 1]]),
                    AP(mx, pb, [[2, T], [1, 1]]),
                ).then_inc(s_v, 1)
                nv += 1
                vector.wait_ge(s_v, nv)
                if b >= 2:
                    vector.wait_ge(s_odma, 16 * (2 * (b - 2) + 2))
                vector.tensor_scalar_mul(
                    AP(hrow, pb * N, [[2 * N, T], [1, N]]),
                    AP(psum_d7s[pb], 0, [[N, T], [1, N]]),
                    AP(rmx, pb, [[2, T], [1, 1]]),
                ).then_inc(s_hr, 1)

        # ================= PE: chains + state-update groups =================
        @block.tensor
        def _(tensor):
            def inject_T(tu, h, s_hrow_h, s_rt_h):
                # state-update group for step tu, half h:
                # ps_rt[h] = 0.85*r_tu[h]  (identity matmul, f32r)
                #          + (ALPHA*relu(rec_tu))[:, h*128:]^T  (bf16 transpose)
                tensor.wait_ge(s_sm, tu + 1)
                tensor.matmul(
                    ps_rt[h].ap(),
                    identr.ap().bitcast(F32R),
                    AP(rt, h * BL, [[P_RT, 128], [1, BL]]).bitcast(F32R),
                    start=True, stop=False,
                )
                tensor.wait_ge(s_hrow_h, tu + 1)
                tensor.matmul(
                    ps_rt[h].ap(),
                    AP(relu_row, h * 128, [[N, BL], [1, 128]]),
                    identb.ap(),
                    start=False, stop=True,
                ).then_inc(s_rt_h, 1)

            tensor.wait_ge(s_boot, 96)
            for t in range(T):
                for m in range(2):
                    for k in range(KCH):
                        if m == 1 and k == INS:
                            inject_T(t, 0, s_hrow0, s_rt0)      # T_h0(t)
                        if m == 0 and k == INS and t >= 1:
                            inject_T(t - 1, 1, s_hrow1, s_rt1)  # T_h1(t-1)
                        h, blk = k // NBLK, k % NBLK
                        if k == 0:
                            tensor.wait_ge(s_st, 2 * t + 1)
                        elif k == NBLK:
                            tensor.wait_ge(s_st, 2 * t + 2)
                        inst = tensor.matmul(
                            AP(ps_rec[m], 0, [[128, BL], [1, 128]]),
                            AP(st, (t % 2) * 2 * NBLK * BL + h * NBLK * BL + blk * BL,
                               [[P_ST, 128], [1, BL]]),
                            AP(wcat, h * NBLK * N + blk * N + m * 128,
                               [[P_WCAT, 128], [1, 128]]),
                            start=(k == 0),
                            stop=(k == KCH - 1),
                        )
                        if k == KCH - 1:
                            inst.then_inc(s_rec, 1)
            inject_T(T - 1, 1, s_hrow1, s_rt1)                  # trailing T_h1(T-1)
            # ---- endgame ----
            tensor.wait_ge(s_up, T)
            for b in range(BL):
                # bump row transposes
                if b >= 1:
                    tensor.wait_ge(s_br, b)  # psum_tb consumed
                for h in range(2):
                    inst = tensor.transpose(
                        AP(psum_tb, h * 128, [[2 * 128, T], [1, 128]]),
                        AP(bumpT, h * BL * T + b * T, [[P_BT, 128], [1, T]]),
                        ident.ap(),
                    )
                    if h == 1:
                        inst.then_inc(s_tb, 1)
                # d7 matmuls
                if b >= 2:
                    tensor.wait_ge(s_hr, b - 1)  # psum_d7 buf consumed
                pb = b % 2
                tensor.matmul(
                    AP(psum_d7s[pb], 0, [[N, T], [1, N]]),
                    AP(bumpT, 0 * BL * T + b * T, [[P_BT, 128], [1, T]]),
                    AP(wd7, 0 * N, [[2 * N, 128], [1, N]]),
                    start=True, stop=False,
                )
                tensor.matmul(
                    AP(psum_d7s[pb], 0, [[N, T], [1, N]]),
                    AP(bumpT, 1 * BL * T + b * T, [[P_BT, 128], [1, T]]),
                    AP(wd7, 1 * N, [[2 * N, 128], [1, N]]),
                    start=False, stop=True,
                ).then_inc(s_d7, 1)

        # ================= ACT: relu(ALPHA * rec) halves -> SBUF row =========
        @block.scalar
        def _(scalar):
            scalar.wait_ge(s_boot, 96)
            for t in range(T):
                scalar.wait_ge(s_rec, 2 * t + 1)
                scalar.activation(
                    AP(relu_row, 0, [[N, BL], [1, 128]]),
                    ps_rec[0].ap(),
                    mybir.ActivationFunctionType.Relu,
                    scale=float(ALPHA),
                ).then_inc(s_hrow0, 1)
                scalar.wait_ge(s_rec, 2 * t + 2)
                scalar.activation(
                    AP(relu_row, 128, [[N, BL], [1, 128]]),
                    ps_rec[1].ap(),
                    mybir.ActivationFunctionType.Relu,
                    scale=float(ALPHA),
                ).then_inc(s_hrow1, 1)
            # ---- endgame: psum_tb -> brow ----
            for b in range(BL):
                scalar.wait_ge(s_tb, b + 1)
                if b >= 2:
                    scalar.wait_ge(s_odma, 16 * (2 * (b - 2) + 1))
                scalar.copy(
                    AP(brow, (b % 2) * N, [[2 * N, T], [1, N]]),
                    AP(psum_tb, 0, [[2 * 128, T], [1, N]]),
                ).then_inc(s_br, 1)

    return nc


def _round_fp32r(x):
    # round-to-nearest-even onto fp32r (1s + 8e + 11m, low 12 bits zero)
    u = x.view(np.uint32)
    u = (u + np.uint32(0x7FF) + ((u >> np.uint32(12)) & np.uint32(1))) & np.uint32(0xFFFFF000)
    return u.view(np.float32)


def _host_prep(action_signal, Wo, Wa, T):
    # Wcat [NBLK, N, N] -> bf16
    wcat = np.empty((NBLK, N, N), dtype=np.float32)
    wcat[:A] = Wa
    wcat[A] = J1 * Wo + J0 * np.ones((N, N), dtype=np.float32)
    wcat = wcat.astype(ml_dtypes.bfloat16)
    # chunk layout [2, NBLK, 128, N]
    wcat_d = np.ascontiguousarray(
        wcat.reshape(NBLK, 2, 128, N).transpose(1, 0, 2, 3))

    # acat [B, T, NBLK]
    acat = np.concatenate(
        [action_signal[:, :T, :],
         np.ones((B, T, 1), dtype=np.float32)], axis=2)

    # r0 row
    idx = np.arange(N, dtype=np.float32)
    center = np.float32(np.pi) * N / (2.0 * np.float32(np.pi))
    d = np.abs(idx - center)
    dist = np.minimum(d, N - d)
    width = N / 10.0
    bump0 = np.exp(-(dist ** 2) / (2.0 * width ** 2)).astype(np.float32)
    bump0 = bump0 / np.float32(np.linalg.norm(bump0))
    r0t = np.ascontiguousarray(
        np.broadcast_to(bump0.reshape(2, 128).T[:, :, None], (128, 2, BL))
    ).astype(np.float32)
    r0t = _round_fp32r(r0t)

    # Wd7 halves
    ii = np.arange(N, dtype=np.float32)
    ang = 2.0 * np.pi * (ii[:, None] - ii[None, :]) / N
    wd7 = np.cos(ang).astype(np.float32)
    wd7_d = np.ascontiguousarray(wd7.reshape(2, 128, N))

    ident = np.eye(128, dtype=np.float32)
    identb = np.eye(8, dtype=ml_dtypes.bfloat16)

    in_maps = []
    for c in range(NC):
        ac_core = np.ascontiguousarray(
            acat[c * BL:(c + 1) * BL].transpose(1, 2, 0).reshape(T, NBLK * BL))
        in_maps.append({
            "wcat": wcat_d, "ac": ac_core, "r0t": r0t,
            "wd7": wd7_d, "ident": ident, "identr": ident, "identb": identb,
        })
    return in_maps


def run(action_signal, Wo, Wa, T=T_FULL, **run_kwargs):
    if T not in _NC_CACHE:
        _NC_CACHE[T] = build_nc(T)
    nc = _NC_CACHE[T]
    in_maps = _host_prep(np.asarray(action_signal, dtype=np.float32),
                         np.asarray(Wo, dtype=np.float32),
                         np.asarray(Wa, dtype=np.float32), T)
    res = run_bass_kernel_spmd(nc, in_maps, core_ids=list(range(NC)), **run_kwargs)
    hist = np.concatenate([r["hist_out"] for r in res.results], axis=0)
    bump = np.concatenate([r["bump_out"] for r in res.results], axis=0)
    return (hist, bump), res


def kernel(action_signal, Wo, Wa):
    (hist, bump), _ = run(action_signal, Wo, Wa, T=T_FULL)
    return hist, bump
